# revision 9
# baseline (speedup 1.0000x reference)
"""Trainium2 Bass kernel for the RNN decoder (heterogeneous 8-core version).

Math (reference):
    tokens = [SOS, target[:,1:]]                       (B, T)
    x      = emb[tokens]                               (B, T, E)
    h_t    = tanh(x_t @ W_ih^T + b_ih + h_{t-1} @ W_hh^T + b_hh)
    out_t  = h_t @ W_out^T + b_out                     (B, V)

Strategy (8 cores, heterogeneous via tc.If(partition_id)):
  - Core 0 runs the sequential 128-step recurrence (the x@W_ih pre-term is
    folded into each step's PSUM via an identity matmul) and broadcasts the
    H history in 16 half-chunks (8 steps, [128,2048] bf16 = 512KB) via
    AllReduce collectives where peers contribute zeros.
  - Vocab is sharded as 250 blocks of 128 rows with NO padding: core 0
    takes 12 blocks (it spends most of its time on the chain), cores 1-7
    take 34 each.
  - Peers keep their W_out shard resident in SBUF, aliased onto core 0's
    H-slab / pre-chunk pool slots via tags (the arms are mutually
    exclusive), and project each half-chunk as it arrives.
  - Collective completion is signalled with an explicit semaphore
    (.then_inc on the collective, sync.wait_ge before readers).
"""

import numpy as np
import ml_dtypes

import concourse.bacc as bacc
import concourse.tile as tile
from concourse import mybir
from concourse.bass_utils import run_bass_kernel_spmd

B, T = 32, 128
E, H, V = 512, 1024, 32000
SOS_IDX = 1
NCORES = 8
BT = B * T            # 4096
NCH = T // 16         # 8 full chunks (pre-compute granularity)
NHC = T // 8          # 16 half-chunks (collective granularity)
NV0 = 12              # vocab blocks on core 0
NVP = 34              # vocab blocks per peer; 12 + 7*34 = 250 = V/128
BF16 = mybir.dt.bfloat16
F32 = mybir.dt.float32
_bf = ml_dtypes.bfloat16

_CACHE = {}


def _build():
    nc = bacc.Bacc(None, target_bir_lowering=False, debug=False,
                   num_devices=NCORES)

    xT_d = nc.dram_tensor("xt", [E, BT], BF16, kind="ExternalInput")
    wih_d = nc.dram_tensor("wih", [E, H], BF16, kind="ExternalInput")
    whh_d = nc.dram_tensor("whh", [H, H], BF16, kind="ExternalInput")
    bsum_d = nc.dram_tensor("bsum", [128, 8], F32, kind="ExternalInput")
    ident_d = nc.dram_tensor("ident", [128, 128], BF16, kind="ExternalInput")
    wout_d = nc.dram_tensor("wout", [NVP, 128, 1024], BF16, kind="ExternalInput")
    bout_d = nc.dram_tensor("bout", [128, NVP], F32, kind="ExternalInput")
    h0t_d = nc.dram_tensor("h0t", [128, 256], BF16, kind="ExternalInput")
    out_d = nc.dram_tensor("out", [NVP * 128, BT], F32, kind="ExternalOutput")

    ADD = mybir.AluOpType.add
    TANH = mybir.ActivationFunctionType.Tanh
    IDENT = mybir.ActivationFunctionType.Identity

    with tile.TileContext(nc) as tc:
        with (
            tc.tile_pool(name="big", bufs=1) as big,
            tc.tile_pool(name="xp", bufs=2) as xp,
            tc.tile_pool(name="wtp", bufs=4) as wtp,
            tc.tile_pool(name="stp", bufs=6) as stp,
            tc.tile_pool(name="tmpp", bufs=2) as tmpp,
            tc.tile_pool(name="dram", bufs=1, space="DRAM") as dram,
            tc.tile_pool(name="psA", bufs=4, space="PSUM") as psA,
            tc.tile_pool(name="psP", bufs=2, space="PSUM") as psP,
            tc.tile_pool(name="psB", bufs=2, space="PSUM") as psB,
        ):
            # ---- shared tile declarations (arms alias via matching tags) --
            # core-0 H history slabs  <-> peers' resident W_out shard
            Hc = [big.tile([128, 4096], BF16, name=f"hc{c}", tag=f"slab{c}")
                  for c in range(NCH)]
            wsl = Hc  # peers view the same 8 slots as W_out storage
            wsl8 = big.tile([128, 4096], BF16, name="wsl8", tag="slab8")
            # core-0 pre-chunk slots  <-> peers' H half-chunk buffers + zero
            prc = [big.tile([128, 4096], BF16, name=f"pre{c}", tag=f"pre{c}")
                   for c in range(3)]
            hgp = [prc[0][:, 0:2048], prc[1][:, 0:2048]]
            zt = prc[2][:, 0:2048]
            whh = big.tile([128, 8 * H], BF16, tag="whh")
            wih = big.tile([128, 4 * H], BF16, tag="wih")
            bsum = big.tile([128, 8], F32, tag="bsum")
            bout = big.tile([128, NVP], F32, tag="bout")
            h0t = big.tile([128, 256], BF16, tag="h0t")
            ident = big.tile([128, 128], BF16, tag="ident")

            cc_in = [dram.tile([128, 2048], BF16, name=f"ccin{h}",
                               tag=f"ccin{h}") for h in range(NHC)]
            cc_out = [dram.tile([128, 2048], BF16, addr_space="Shared",
                                name=f"ccout{h}", tag=f"ccout{h}")
                      for h in range(NHC)]

            # bout used by every core
            nc.sync.dma_start(bout[:], bout_d[:])

            def emit_pre_block(c, ho, xc_box):
                """pre chunk c, output block ho: 4 matmuls + DVE bias add."""
                slot = prc[c % 3]
                if ho == 0:
                    xc = xp.tile([128, 2048], BF16, name="xc", tag="xc")
                    xc_box[0] = xc
                    for e in range(4):
                        nc.sync.dma_start(
                            xc[:, e * 512:(e + 1) * 512],
                            xT_d[e * 128:(e + 1) * 128,
                                 c * 512:(c + 1) * 512])
                xc = xc_box[0]
                dst3 = slot[:].rearrange("p (t q) -> p t q", q=256)
                acc = psP.tile([128, 512], F32)
                for e in range(4):
                    nc.tensor.matmul(
                        acc[:],
                        wih[:, e * H + ho * 128: e * H + ho * 128 + 128],
                        xc[:, e * 512:(e + 1) * 512],
                        start=(e == 0), stop=(e == 3))
                nc.vector.tensor_scalar(
                    dst3[:, :, ho * 32:(ho + 1) * 32],
                    acc[:].rearrange("p (t b) -> p t b", b=32),
                    bsum[:, ho:ho + 1], None, op0=ADD)

            def emit_step(t):
                c, tl = t // 16, t % 16
                ps = psB.tile([128, 256], F32)
                if t == 0:
                    hprev = h0t[:]
                else:
                    hprev = Hc[(t - 1) // 16][:, ((t - 1) % 16) * 256:
                                              ((t - 1) % 16) * 256 + 256]
                for ho in range(8):
                    seg = ps[:, ho * 32:(ho + 1) * 32]
                    for kh in range(8):
                        nc.tensor.matmul(
                            seg,
                            whh[:, kh * H + ho * 128: kh * H + ho * 128 + 128],
                            hprev[:, kh * 32:(kh + 1) * 32],
                            start=(kh == 0), stop=(kh == 7))
                tmp = tmpp.tile([128, 256], F32)
                nc.vector.tensor_tensor(
                    tmp[:], ps[:], prc[c % 3][:, tl * 256:(tl + 1) * 256],
                    op=ADD)
                nc.scalar.activation(Hc[c][:, tl * 256:(tl + 1) * 256],
                                     tmp[:], TANH)

            pid = nc.partition_id()
            with tc.If(pid == 0) as cif:
                # ---------------- core 0: recurrence chain ----------------
                for e in range(4):
                    nc.sync.dma_start(wih[:, e * H:(e + 1) * H],
                                      wih_d[e * 128:(e + 1) * 128, :])
                nc.sync.dma_start(bsum[:], bsum_d[:])
                nc.sync.dma_start(h0t[:], h0t_d[:])
                nc.sync.dma_start(ident[:], ident_d[:])
                for kh in range(8):
                    nc.sync.dma_start(whh[:, kh * H:(kh + 1) * H],
                                      whh_d[kh * 128:(kh + 1) * 128, :])

                xc_box = [None]
                for ho in range(8):
                    emit_pre_block(0, ho, xc_box)
                for ho in range(8):
                    emit_pre_block(1, ho, xc_box)
                for t in range(T):
                    c, tl = t // 16, t % 16
                    if tl < 8 and c + 2 < NCH:
                        emit_pre_block(c + 2, tl, xc_box)
                    emit_step(t)
                    if t % 8 == 7:
                        h = t // 8
                        nc.sync.dma_start(
                            cc_in[h][:],
                            Hc[c][:, (h % 2) * 2048:(h % 2) * 2048 + 2048])
            with cif.Else():
                # -------- peers: zero collective inputs, load W_out -------
                nc.gpsimd.memset(zt, 0.0)
                for h in range(NHC):
                    nc.sync.dma_start(cc_in[h][:], zt)
                for v in range(NVP):
                    dst = wsl8 if v // 4 == 8 else wsl[v // 4]
                    nc.sync.dma_start(
                        dst[:, (v % 4) * 1024:(v % 4) * 1024 + 1024],
                        wout_d[v])

            # ---------------- broadcast H (all cores, in order) -----------
            for h in range(NHC):
                nc.gpsimd.collective_compute(
                    "AllReduce", ADD,
                    replica_groups=[list(range(NCORES))],
                    ins=[cc_in[h].opt()],
                    outs=[cc_out[h].opt()],
                )

            def emit_proj(hg, h, v, wt):
                """project half-chunk h (8 steps) for vocab block v."""
                rhs3 = hg.rearrange("p (t q) -> p t q", q=256)
                acc = psA.tile([128, 256], F32)
                for kh in range(8):
                    nc.tensor.matmul(
                        acc[:],
                        wt[:, kh * 128:(kh + 1) * 128],
                        rhs3[:, :, kh * 32:(kh + 1) * 32],
                        start=(kh == 0), stop=(kh == 7))
                sg = stp.tile([128, 256], F32)
                nc.scalar.activation(sg[:], acc[:], IDENT,
                                     bias=bout[:, v:v + 1])
                nc.sync.dma_start(
                    out_d[v * 128:(v + 1) * 128, h * 256:(h + 1) * 256],
                    sg[:])

            pid2 = nc.partition_id()
            with tc.If(pid2 == 0) as cif2:
                # core 0: 12 blocks, W_out streamed, H read back from DRAM
                # (the prc slots are dead once the chain is done)
                for h in range(NHC):
                    hg = Hc[h // 2][:, (h % 2) * 2048:(h % 2) * 2048 + 2048]
                    for v in range(NV0):
                        wt = wtp.tile([128, 1024], BF16, name="wt", tag="wt")
                        nc.sync.dma_start(wt[:], wout_d[v])
                        emit_proj(hg, h, v, wt[:])
            with cif2.Else():
                for h in range(NHC):
                    hg = hgp[h % 2]
                    nc.gpsimd.dma_start(hg, cc_out[h][:])
                    for v in range(NVP):
                        wt = (wsl8 if v // 4 == 8 else wsl[v // 4])[
                            :, (v % 4) * 1024:(v % 4) * 1024 + 1024]
                        emit_proj(hg, h, v, wt)
    nc.compile()
    return nc


def _get_nc():
    if "nc" not in _CACHE:
        _CACHE["nc"] = _build()
    return _CACHE["nc"]


def _prep_inputs(target, h0, emb, W_ih, b_ih, W_hh, b_hh, W_out, b_out):
    target = np.asarray(target)
    h0 = np.asarray(h0, dtype=np.float32)
    emb = np.asarray(emb, dtype=np.float32)
    W_ih = np.asarray(W_ih, dtype=np.float32)
    b_ih = np.asarray(b_ih, dtype=np.float32)
    W_hh = np.asarray(W_hh, dtype=np.float32)
    b_hh = np.asarray(b_hh, dtype=np.float32)
    W_out = np.asarray(W_out, dtype=np.float32)
    b_out = np.asarray(b_out, dtype=np.float32)

    tokens = np.concatenate(
        [np.full((B, 1), SOS_IDX, dtype=target.dtype), target[:, 1:]], axis=1)
    x = emb[tokens]                                   # (B, T, E) f32
    # xT[e, t*B + b] = x[b, t, e]
    xT = np.ascontiguousarray(x.transpose(2, 1, 0).reshape(E, BT)).astype(_bf)
    wihT = np.ascontiguousarray(W_ih.T).astype(_bf)   # (E, H)
    whhT = np.ascontiguousarray(W_hh.T).astype(_bf)   # (H, H)
    # bsum[p, ho] = (b_ih + b_hh)[ho*128 + p]
    bsum = np.ascontiguousarray((b_ih + b_hh).reshape(8, 128).T)
    # h0t[p, kh*32 + b] = h0[b, kh*128 + p]
    h0t = np.ascontiguousarray(
        h0.reshape(B, 8, 128).transpose(2, 1, 0).reshape(128, 256)).astype(_bf)
    ident = np.eye(128, dtype=np.float32).astype(_bf)

    # vocab block assignment: core 0 -> blocks [0, NV0); peer c -> blocks
    # [NV0 + (c-1)*NVP, ... + NVP).  250 blocks of 128 rows = 32000 exactly.
    shared = dict(xt=xT, wih=wihT, whh=whhT, bsum=bsum, h0t=h0t, ident=ident)
    in_maps = []
    for c in range(NCORES):
        nvb = NV0 if c == 0 else NVP
        r0 = 0 if c == 0 else (NV0 + (c - 1) * NVP) * 128
        ws = W_out[r0:r0 + nvb * 128]                 # (nvb*128, 1024)
        # wout[vb, p, kh*128 + m] = ws[vb*128 + m, kh*128 + p]
        wr = np.zeros((NVP, 128, 1024), dtype=_bf)
        wr[:nvb] = np.ascontiguousarray(
            ws.reshape(nvb, 128, 8, 128).transpose(0, 3, 2, 1)
            .reshape(nvb, 128, 1024)).astype(_bf)
        bs = np.zeros((128, NVP), dtype=np.float32)
        bs[:, :nvb] = b_out[r0:r0 + nvb * 128].reshape(nvb, 128).T
        in_maps.append(dict(shared, wout=wr, bout=np.ascontiguousarray(bs)))
    return in_maps


def kernel(target, h0, emb, W_ih, b_ih, W_hh, b_hh, W_out, b_out):
    nc = _get_nc()
    in_maps = _prep_inputs(target, h0, emb, W_ih, b_ih, W_hh, b_hh, W_out, b_out)
    _CACHE["last_in_maps"] = in_maps
    res = run_bass_kernel_spmd(nc, in_maps, core_ids=list(range(NCORES)))
    _CACHE["last_result"] = res
    full = np.empty((V, BT), dtype=np.float32)
    full[0:NV0 * 128] = res.results[0]["out"][0:NV0 * 128]
    for c in range(1, NCORES):
        r0 = (NV0 + (c - 1) * NVP) * 128
        full[r0:r0 + NVP * 128] = res.results[c]["out"]
    out = full.reshape(V, T, B).transpose(2, 1, 0)    # (B, T, V)
    return np.ascontiguousarray(out)
